# revision 30
# baseline (speedup 1.0000x reference)
"""CRZ diagonal-unitary kernel for Trainium2 (8 NeuronCores).

The reference computes U @ x where U = diag(d), d[n] a phase that depends only
on the top two bits of the row index n (D = 4096 rows, DIM=2, WIRES=12,
control wire 0, target wire 1, J=1):
  rows [0, 2048)    : phase = 1                      (control digit 0)
  rows [2048, 3072) : phase = exp(-i * angle/2)      (control 1, target 0)
  rows [3072, 4096) : phase = exp(+i * angle/2)      (control 1, target 1)

So the whole op is an elementwise per-row complex scalar multiply - purely
memory bound.  Sharding: rows across the 8 cores (512 rows each, fully
contiguous DRAM slices; each core's phase is a single (a, b, d) coefficient
triple passed as a tiny input tensor so one SPMD program serves all cores):
  out_r = a*xr + b*xi
  out_i = a*xi + d*xr
The kernel writes the interleaved complex64 layout directly (f32 pairs).

Raw Bass (no TileContext): the Tile layer's multi-wait drain instructions are
rejected by this walrus build ("Too many sync wait commands").

The execution backend here charges a large fixed cost per instruction and per
blocked semaphore wait (~40-70 us each) while data size barely matters, so the
default variant (v11) minimizes instructions: host packs [xi ; xr] into one
input, the device runs 1 load DMA + 2 whole-slice DVE ops (a tensor_tensor
prefill of both interleaved complex planes via a stride-0-broadcast (b,d)
pattern, then one aliasing scalar_tensor_tensor accumulate via a negative-
stride half-swapped view) + 1 store DMA, with 2 blocked waits total.
Measured ~0.29 ms/invocation per core (repetition-slope method) vs ~1.6 ms
for a classic 4-tile double-buffered pipeline (v1); cost-model (TimelineSim)
time 86.6 us vs the ~47 us pure-DMA roofline.
"""

import math

import numpy as np

import concourse.bass as bass
import concourse.mybir as mybir
from concourse.bass_utils import run_bass_kernel_spmd

D = 4096
BATCH = 2048
N_CORES = 8
ROWS = D // N_CORES  # 512 rows per core
P = 128              # SBUF partitions
NT = ROWS // P       # row tiles per core (4)
NBUF = 2

VARIANT = "v11"      # which _build variant kernel() uses

_NC_CACHE = {}


def _io(nc, bench):
    f32 = mybir.dt.float32
    big_kind = "Internal" if bench else None
    xr = nc.dram_tensor("xr", [ROWS, BATCH], f32, kind=big_kind or "ExternalInput")
    xi = nc.dram_tensor("xi", [ROWS, BATCH], f32, kind=big_kind or "ExternalInput")
    coef = nc.dram_tensor("coef", [P, 3], f32, kind="ExternalInput")
    out = nc.dram_tensor("out", [ROWS, 2 * BATCH], f32, kind=big_kind or "ExternalOutput")
    out_small = None
    if bench:
        out_small = nc.dram_tensor("out_small", [P, 3], f32, kind="ExternalOutput")
    return xr, xi, coef, out, out_small


def _build(reps=1, bench=False, variant=None):
    """Build the per-core Bass program.

    reps > 1 repeats the body (same data) inside one NEFF - benchmarking only.
    bench=True makes the big tensors Internal DRAM scratch (garbage data,
    identical instruction stream) so per-call transfer cost vanishes.
    """
    variant = variant or VARIANT
    key = (reps, bench, variant)
    if key in _NC_CACHE:
        return _NC_CACHE[key]
    nc = {
        "v1": _build_v1,
        "v2": _build_v2,
        "v3": _build_v3,
        "v4": _build_v4,
        "v5": _build_v5,
        "v6": _build_v6,
        "v9": _build_v9,
        "v10": _build_v10,
        "v11": _build_v11,
        "v13": _build_v13,
    }[variant](reps, bench)
    _NC_CACHE[key] = nc
    return nc


def _build_v13(reps, bench):
    """v11 with both big DMAs split into two parallel halves (SP + ACT).
    10 instructions, 3 blocked waits, ~4 MB per DMA."""
    f32 = mybir.dt.float32
    mult = mybir.AluOpType.mult
    add = mybir.AluOpType.add

    nc = bass.Bass()
    big_kind = "Internal" if bench else None
    xin = nc.dram_tensor("xin", [2 * ROWS, BATCH], f32, kind=big_kind or "ExternalInput")
    coef = nc.dram_tensor("coef", [P, 3], f32, kind="ExternalInput")
    out = nc.dram_tensor("out", [ROWS, 2 * BATCH], f32, kind=big_kind or "ExternalOutput")
    if bench:
        out_small = nc.dram_tensor("out_small", [P, 3], f32, kind="ExternalOutput")

    xin_v = xin[:, :].rearrange("(t p) w -> p t w", p=P)    # t = 0..7
    out_v = out[:, :].rearrange("(t p) w -> p t w", p=P)

    H = NT * BATCH  # 8192 elements per half per partition

    with (
        nc.sbuf_tensor([P, 2 * H], f32) as xin_b,
        nc.sbuf_tensor([P, 2 * H], f32) as out_b,
        nc.sbuf_tensor([P, 3], f32) as coef_t,
        nc.semaphore() as ld_sem,
        nc.semaphore() as dve_sem,
        nc.semaphore() as st_sem,
        nc.Block() as block,
    ):
        xin_b3 = xin_b[:, :].rearrange("p (t w) -> p t w", t=2 * NT)
        out_b3 = out_b[:, :].rearrange("p (t w) -> p t w", t=NT)

        xin3 = xin_b[:, :].rearrange("p (j k) -> p j k", j=2)
        xin3_swap = bass.AP(
            tensor=xin3.tensor,
            offset=xin3.offset + H,
            ap=[list(xin3.ap[0]), [-H, 2], list(xin3.ap[2])],
        )
        out3 = out_b[:, :].rearrange("p (k j) -> p j k", j=2)

        a_ap = coef_t[:, 0:1]
        bd_pat = (
            coef_t[:, 1:3]
            .rearrange("p (j o) -> p j o", j=2)
            .broadcast_to((P, 2, H))
        )

        @block.sync
        def _(sync):
            for r in range(reps):
                sync.dma_start(xin_b3[:, 0:NT, :], xin_v[:, 0:NT, :]).then_inc(
                    ld_sem, 16
                )
                sync.wait_ge(dve_sem, 2 * (r + 1))
                sync.dma_start(out_v[:, 0:2, :], out_b3[:, 0:2, :]).then_inc(
                    st_sem, 16
                )
            if bench:
                sync.wait_ge(st_sem, 32 * reps)
                sync.dma_start(out_small[:, :], coef_t[:, :]).then_inc(st_sem, 16)

        @block.scalar
        def _(scalar):
            scalar.dma_start(coef_t[:, :], coef[:, :]).then_inc(ld_sem, 16)
            for r in range(reps):
                if r:
                    scalar.wait_ge(dve_sem, 2 * r)  # xin_b still read by DVE
                scalar.dma_start(
                    xin_b3[:, NT : 2 * NT, :], xin_v[:, NT : 2 * NT, :]
                ).then_inc(ld_sem, 16)
                scalar.wait_ge(dve_sem, 2 * (r + 1))
                scalar.dma_start(out_v[:, 2:NT, :], out_b3[:, 2:NT, :]).then_inc(
                    st_sem, 16
                )

        @block.vector
        def _(vector):
            for r in range(reps):
                vector.wait_ge(ld_sem, 16 + 32 * (r + 1))
                if r:
                    vector.wait_ge(st_sem, 32 * r)  # out_b free again
                nc.vector.tensor_tensor(out3, xin3, bd_pat, op=mult).then_inc(
                    dve_sem, 1
                )
                nc.vector.scalar_tensor_tensor(
                    out3, xin3_swap, a_ap, out3, op0=mult, op1=add
                ).then_inc(dve_sem, 1)

    return nc


def _build_v11(reps, bench):
    """Two-compute-op variant: 7 instructions, 2 blocked waits.

    Host packs one [2*ROWS, BATCH] input: rows 0..511 = xi, rows 512..1023 =
    xr.  In SBUF that is [128, 16384] with the xi half at [0:8192] and the xr
    half at [8192:16384] per partition, so 3-D access patterns let ONE
    tensor_tensor prefill both interleaved output planes ([b*xi | d*xr] via a
    stride-0-broadcast (b,d) pattern) and ONE scalar_tensor_tensor accumulate
    ([+a*xr | +a*xi] via a negative-stride half-swapped view).  The tiny coef
    load rides ACT, off the critical path.
    """
    f32 = mybir.dt.float32
    mult = mybir.AluOpType.mult
    add = mybir.AluOpType.add

    nc = bass.Bass()
    big_kind = "Internal" if bench else None
    xin = nc.dram_tensor("xin", [2 * ROWS, BATCH], f32, kind=big_kind or "ExternalInput")
    coef = nc.dram_tensor("coef", [P, 3], f32, kind="ExternalInput")
    out = nc.dram_tensor("out", [ROWS, 2 * BATCH], f32, kind=big_kind or "ExternalOutput")
    if bench:
        out_small = nc.dram_tensor("out_small", [P, 3], f32, kind="ExternalOutput")

    xin_v = xin[:, :].rearrange("(t p) w -> p t w", p=P)    # t = 0..7
    out_v = out[:, :].rearrange("(t p) w -> p t w", p=P)

    H = NT * BATCH  # 8192 elements per half per partition

    with (
        nc.sbuf_tensor([P, 2 * H], f32) as xin_b,
        nc.sbuf_tensor([P, 2 * H], f32) as out_b,
        nc.sbuf_tensor([P, 3], f32) as coef_t,
        nc.semaphore() as ld_sem,
        nc.semaphore() as dve_sem,
        nc.semaphore() as st_sem,
        nc.Block() as block,
    ):
        xin_b3 = xin_b[:, :].rearrange("p (t w) -> p t w", t=2 * NT)
        out_b3 = out_b[:, :].rearrange("p (t w) -> p t w", t=NT)

        # [P, 2, H]: j selects the xi/xr half
        xin3 = xin_b[:, :].rearrange("p (j k) -> p j k", j=2)
        # half-swapped view ([xr | xi]): j step negated from offset H
        xin3_swap = bass.AP(
            tensor=xin3.tensor,
            offset=xin3.offset + H,
            ap=[list(xin3.ap[0]), [-H, 2], list(xin3.ap[2])],
        )
        # output as [P, 2(plane), H]: plane index j is innermost in memory
        out3 = out_b[:, :].rearrange("p (k j) -> p j k", j=2)

        a_ap = coef_t[:, 0:1]
        bd_pat = (
            coef_t[:, 1:3]
            .rearrange("p (j o) -> p j o", j=2)
            .broadcast_to((P, 2, H))
        )

        @block.scalar
        def _(scalar):
            scalar.dma_start(coef_t[:, :], coef[:, :]).then_inc(ld_sem, 16)

        @block.sync
        def _(sync):
            for r in range(reps):
                sync.dma_start(xin_b3, xin_v).then_inc(ld_sem, 16)
                sync.wait_ge(dve_sem, 2 * (r + 1))
                sync.dma_start(out_v, out_b3).then_inc(st_sem, 16)
            if bench:
                sync.wait_ge(st_sem, 16 * reps)
                sync.dma_start(out_small[:, :], coef_t[:, :]).then_inc(st_sem, 16)

        @block.vector
        def _(vector):
            for r in range(reps):
                vector.wait_ge(ld_sem, 16 + 16 * (r + 1))
                if r:
                    vector.wait_ge(st_sem, 16 * r)  # out_b free again
                nc.vector.tensor_tensor(out3, xin3, bd_pat, op=mult).then_inc(
                    dve_sem, 1
                )
                nc.vector.scalar_tensor_tensor(
                    out3, xin3_swap, a_ap, out3, op0=mult, op1=add
                ).then_inc(dve_sem, 1)

    return nc


WA = BATCH + 4  # xr row width with (a, b, d, pad) appended


def _build_v9(reps, bench, split_store=False):
    """Coefficients ride as 4 extra columns on xr (host-packed), so the
    whole kernel is: 2 loads (SP: xr+coef, ACT: xi), 4 DVE ops, 1 store.
    9 instructions, 2 blocked waits (10/3 with split_store)."""
    f32 = mybir.dt.float32
    mult = mybir.AluOpType.mult
    add = mybir.AluOpType.add

    nc = bass.Bass()
    big_kind = "Internal" if bench else None
    xr = nc.dram_tensor("xr", [ROWS, WA], f32, kind=big_kind or "ExternalInput")
    xi = nc.dram_tensor("xi", [ROWS, BATCH], f32, kind=big_kind or "ExternalInput")
    out = nc.dram_tensor("out", [ROWS, 2 * BATCH], f32, kind=big_kind or "ExternalOutput")
    out_small = None
    if bench:
        # bench still needs one tiny real input/output pair
        coef_in = nc.dram_tensor("coef", [P, 3], f32, kind="ExternalInput")
        out_small = nc.dram_tensor("out_small", [P, 3], f32, kind="ExternalOutput")

    xr_v = xr[:, :].rearrange("(t p) w -> p t w", p=P)
    xi_v = xi[:, :].rearrange("(t p) w -> p t w", p=P)
    out_v = out[:, :].rearrange("(t p) w -> p t w", p=P)

    with (
        nc.sbuf_tensor([P, NT * WA], f32) as xr_b,
        nc.sbuf_tensor([P, NT * BATCH], f32) as xi_b,
        nc.sbuf_tensor([P, 2 * NT * BATCH], f32) as out_b,
        nc.sbuf_tensor([P, 3], f32) as mark,
        nc.semaphore() as ld_sem,
        nc.semaphore() as dve_sem,
        nc.semaphore() as st_sem,
        nc.Block() as block,
    ):
        xr_b3 = xr_b[:, :].rearrange("p (t w) -> p t w", t=NT)
        xi_b3 = xi_b[:, :].rearrange("p (t w) -> p t w", t=NT)
        out_b3 = out_b[:, :].rearrange("p (t w) -> p t w", t=NT)
        xr3 = xr_b3[:, :, 0:BATCH]           # [P, NT, BATCH] data part
        a_ap = xr_b[:, BATCH : BATCH + 1]    # t=0 chunk carries the coefs
        b_ap = xr_b[:, BATCH + 1 : BATCH + 2]
        d_ap = xr_b[:, BATCH + 2 : BATCH + 3]
        o_ev = out_b3[:, :, 0::2]            # [P, NT, BATCH]
        o_od = out_b3[:, :, 1::2]
        HALF = BATCH  # split point of the store in w2 units

        @block.sync
        def _(sync):
            if bench:
                sync.dma_start(mark[:, :], coef_in[:, :]).then_inc(ld_sem, 16)
            for r in range(reps):
                sync.dma_start(xr_b3, xr_v).then_inc(ld_sem, 16)
                sync.wait_ge(dve_sem, 4 * (r + 1))
                if split_store:
                    sync.dma_start(
                        out_v[:, :, :HALF], out_b3[:, :, :HALF]
                    ).then_inc(st_sem, 16)
                else:
                    sync.dma_start(out_v, out_b3).then_inc(st_sem, 16)
            if bench:
                sync.wait_ge(st_sem, 16 * reps * (2 if split_store else 1))
                sync.dma_start(out_small[:, :], mark[:, :]).then_inc(st_sem, 16)

        @block.scalar
        def _(scalar):
            for r in range(reps):
                if r:
                    scalar.wait_ge(dve_sem, 4 * r)  # xi_b still read by DVE
                scalar.dma_start(xi_b3, xi_v).then_inc(ld_sem, 16)
                if split_store:
                    scalar.wait_ge(dve_sem, 4 * (r + 1))
                    scalar.dma_start(
                        out_v[:, :, HALF:], out_b3[:, :, HALF:]
                    ).then_inc(st_sem, 16)

        @block.vector
        def _(vector):
            base = 16 if bench else 0
            for r in range(reps):
                vector.wait_ge(ld_sem, base + 32 * (r + 1))
                if r:
                    nst = 2 if split_store else 1
                    vector.wait_ge(st_sem, 16 * nst * r)  # out_b free again
                nc.vector.tensor_scalar_mul(o_ev, xi_b3, b_ap).then_inc(dve_sem, 1)
                nc.vector.scalar_tensor_tensor(
                    o_ev, xr3, a_ap, o_ev, op0=mult, op1=add
                ).then_inc(dve_sem, 1)
                nc.vector.tensor_scalar_mul(o_od, xr3, d_ap).then_inc(dve_sem, 1)
                nc.vector.scalar_tensor_tensor(
                    o_od, xi_b3, a_ap, o_od, op0=mult, op1=add
                ).then_inc(dve_sem, 1)

    return nc


def _build_v10(reps, bench):
    return _build_v9(reps, bench, split_store=True)


def _common_io_views(nc, bench):
    f32 = mybir.dt.float32
    xr, xi, coef, out, out_small = _io(nc, bench)
    xr_v = xr[:, :].rearrange("(t p) w -> p t w", p=P)
    xi_v = xi[:, :].rearrange("(t p) w -> p t w", p=P)
    out_v = out[:, :].rearrange("(t p) w -> p t w", p=P)
    return coef, out_small, xr_v, xi_v, out_v


def _build_v5(reps, bench):
    """10 instructions, 2 blocked waits: SP loads xr + stores, ACT loads
    coef + xi, DVE does all four compute ops (prefill + aliasing STT)."""
    f32 = mybir.dt.float32
    mult = mybir.AluOpType.mult
    add = mybir.AluOpType.add

    nc = bass.Bass()
    coef, out_small, xr_v, xi_v, out_v = _common_io_views(nc, bench)
    W = NT * BATCH

    with (
        nc.sbuf_tensor([P, 3], f32) as coef_t,
        nc.sbuf_tensor([P, W], f32) as xr_b,
        nc.sbuf_tensor([P, W], f32) as xi_b,
        nc.sbuf_tensor([P, 2 * W], f32) as out_b,
        nc.semaphore() as ld_sem,
        nc.semaphore() as dve_sem,
        nc.semaphore() as st_sem,
        nc.Block() as block,
    ):
        a_ap = coef_t[:, 0:1]
        b_ap = coef_t[:, 1:2]
        d_ap = coef_t[:, 2:3]
        o_ev = out_b[:, 0::2]
        o_od = out_b[:, 1::2]
        xr_b3 = xr_b[:, :].rearrange("p (t w) -> p t w", t=NT)
        xi_b3 = xi_b[:, :].rearrange("p (t w) -> p t w", t=NT)
        out_b3 = out_b[:, :].rearrange("p (t w) -> p t w", t=NT)

        @block.sync
        def _(sync):
            for r in range(reps):
                sync.dma_start(xr_b3, xr_v).then_inc(ld_sem, 16)
                sync.wait_ge(dve_sem, 4 * (r + 1))
                sync.dma_start(out_v, out_b3).then_inc(st_sem, 16)
            if bench:
                sync.wait_ge(st_sem, 16 * reps)
                sync.dma_start(out_small[:, :], coef_t[:, :]).then_inc(st_sem, 16)

        @block.scalar
        def _(scalar):
            scalar.dma_start(coef_t[:, :], coef[:, :]).then_inc(ld_sem, 16)
            for r in range(reps):
                if r:
                    scalar.wait_ge(dve_sem, 4 * r)  # xi_b still read by DVE
                scalar.dma_start(xi_b3, xi_v).then_inc(ld_sem, 16)

        @block.vector
        def _(vector):
            for r in range(reps):
                vector.wait_ge(ld_sem, 16 + 32 * (r + 1))
                if r:
                    vector.wait_ge(st_sem, 16 * r)  # out_b free again
                nc.vector.tensor_scalar_mul(o_ev, xi_b[:, :], b_ap).then_inc(dve_sem, 1)
                nc.vector.scalar_tensor_tensor(
                    o_ev, xr_b[:, :], a_ap, o_ev, op0=mult, op1=add
                ).then_inc(dve_sem, 1)
                nc.vector.tensor_scalar_mul(o_od, xr_b[:, :], d_ap).then_inc(dve_sem, 1)
                nc.vector.scalar_tensor_tensor(
                    o_od, xi_b[:, :], a_ap, o_od, op0=mult, op1=add
                ).then_inc(dve_sem, 1)

    return nc


def _build_v6(reps, bench):
    """12 instructions: loads split SP/ACT, prefills on ACT, STTs on DVE."""
    f32 = mybir.dt.float32
    mult = mybir.AluOpType.mult
    add = mybir.AluOpType.add

    nc = bass.Bass()
    coef, out_small, xr_v, xi_v, out_v = _common_io_views(nc, bench)
    W = NT * BATCH

    with (
        nc.sbuf_tensor([P, 3], f32) as coef_t,
        nc.sbuf_tensor([P, W], f32) as xr_b,
        nc.sbuf_tensor([P, W], f32) as xi_b,
        nc.sbuf_tensor([P, 2 * W], f32) as out_b,
        nc.semaphore() as ld_sem,
        nc.semaphore() as act_sem,
        nc.semaphore() as dve_sem,
        nc.semaphore() as st_sem,
        nc.Block() as block,
    ):
        a_ap = coef_t[:, 0:1]
        b_ap = coef_t[:, 1:2]
        d_ap = coef_t[:, 2:3]
        o_ev = out_b[:, 0::2]
        o_od = out_b[:, 1::2]
        xr_b3 = xr_b[:, :].rearrange("p (t w) -> p t w", t=NT)
        xi_b3 = xi_b[:, :].rearrange("p (t w) -> p t w", t=NT)
        out_b3 = out_b[:, :].rearrange("p (t w) -> p t w", t=NT)

        @block.sync
        def _(sync):
            for r in range(reps):
                sync.dma_start(xr_b3, xr_v).then_inc(ld_sem, 16)
                sync.wait_ge(dve_sem, 2 * (r + 1))
                sync.dma_start(out_v, out_b3).then_inc(st_sem, 16)
            if bench:
                sync.wait_ge(st_sem, 16 * reps)
                sync.dma_start(out_small[:, :], coef_t[:, :]).then_inc(st_sem, 16)

        @block.scalar
        def _(scalar):
            scalar.dma_start(coef_t[:, :], coef[:, :]).then_inc(ld_sem, 16)
            for r in range(reps):
                if r:
                    scalar.wait_ge(dve_sem, 2 * r)  # xi_b still read by DVE
                scalar.dma_start(xi_b3, xi_v).then_inc(ld_sem, 16)
                scalar.wait_ge(ld_sem, 16 + 32 * (r + 1))
                if r:
                    scalar.wait_ge(st_sem, 16 * r)  # out_b free again
                nc.scalar.mul(o_ev, xi_b[:, :], mul=b_ap).then_inc(act_sem, 1)
                nc.scalar.mul(o_od, xr_b[:, :], mul=d_ap).then_inc(act_sem, 1)

        @block.vector
        def _(vector):
            for r in range(reps):
                vector.wait_ge(act_sem, 2 * r + 1)
                nc.vector.scalar_tensor_tensor(
                    o_ev, xr_b[:, :], a_ap, o_ev, op0=mult, op1=add
                ).then_inc(dve_sem, 1)
                vector.wait_ge(act_sem, 2 * r + 2)
                nc.vector.scalar_tensor_tensor(
                    o_od, xi_b[:, :], a_ap, o_od, op0=mult, op1=add
                ).then_inc(dve_sem, 1)

    return nc


def _build_v4(reps, bench):
    """Four-engine minimal-critical-path variant.

    All three loads issue in parallel (SP: xr, ACT: xi, POOL: coef), the two
    interleaved-plane prefills run in parallel (ACT: even, POOL: odd), DVE
    does the two fused accumulating STTs, SP stores.
      13 instructions, 4 blocked waits per invocation.
    """
    f32 = mybir.dt.float32
    mult = mybir.AluOpType.mult
    add = mybir.AluOpType.add

    nc = bass.Bass()
    xr, xi, coef, out, out_small = _io(nc, bench)

    W = NT * BATCH
    xr_v = xr[:, :].rearrange("(t p) w -> p t w", p=P)
    xi_v = xi[:, :].rearrange("(t p) w -> p t w", p=P)
    out_v = out[:, :].rearrange("(t p) w -> p t w", p=P)

    with (
        nc.sbuf_tensor([P, 3], f32) as coef_t,
        nc.sbuf_tensor([P, W], f32) as xr_b,
        nc.sbuf_tensor([P, W], f32) as xi_b,
        nc.sbuf_tensor([P, 2 * W], f32) as out_b,
        nc.semaphore() as ld_sem,     # +16 per load DMA (3 per rep)
        nc.semaphore() as act_sem,    # +1 per prefill (ACT and POOL)
        nc.semaphore() as dve_sem,    # +1 per DVE STT
        nc.semaphore() as st_sem,     # +16 per store
        nc.Block() as block,
    ):
        a_ap = coef_t[:, 0:1]
        b_ap = coef_t[:, 1:2]
        d_ap = coef_t[:, 2:3]
        o_ev = out_b[:, 0::2]
        o_od = out_b[:, 1::2]
        xr_b3 = xr_b[:, :].rearrange("p (t w) -> p t w", t=NT)
        xi_b3 = xi_b[:, :].rearrange("p (t w) -> p t w", t=NT)
        out_b3 = out_b[:, :].rearrange("p (t w) -> p t w", t=NT)

        def ld_after(r):  # ld_sem once rep r's loads are done (coef loads once)
            return 16 + 32 * (r + 1)

        @block.sync
        def _(sync):
            for r in range(reps):
                if r:
                    # xr_b overwrite needs rep r-1's STTs done; store r-1
                    # precedes in program order and already waited for them
                    pass
                sync.dma_start(xr_b3, xr_v).then_inc(ld_sem, 16)
                sync.wait_ge(dve_sem, 2 * (r + 1))
                sync.dma_start(out_v, out_b3).then_inc(st_sem, 16)
            if bench:
                sync.wait_ge(st_sem, 16 * reps)
                sync.dma_start(out_small[:, :], coef_t[:, :]).then_inc(st_sem, 16)

        @block.scalar
        def _(scalar):
            for r in range(reps):
                if r:
                    scalar.wait_ge(dve_sem, 2 * r)  # xi_b still read by STTs
                scalar.dma_start(xi_b3, xi_v).then_inc(ld_sem, 16)
                scalar.wait_ge(ld_sem, ld_after(r))
                if r:
                    scalar.wait_ge(st_sem, 16 * r)  # out_b free again
                nc.scalar.mul(o_ev, xi_b[:, :], mul=b_ap).then_inc(act_sem, 1)

        @block.gpsimd
        def _(g):
            g.dma_start(coef_t[:, :], coef[:, :]).then_inc(ld_sem, 16)
            for r in range(reps):
                g.wait_ge(ld_sem, ld_after(r))
                if r:
                    g.wait_ge(st_sem, 16 * r)
                nc.gpsimd.tensor_scalar_mul(o_od, xr_b[:, :], d_ap).then_inc(act_sem, 1)

        @block.vector
        def _(vector):
            for r in range(reps):
                vector.wait_ge(act_sem, 2 * (r + 1))
                nc.vector.scalar_tensor_tensor(
                    o_ev, xr_b[:, :], a_ap, o_ev, op0=mult, op1=add
                ).then_inc(dve_sem, 1)
                nc.vector.scalar_tensor_tensor(
                    o_od, xi_b[:, :], a_ap, o_od, op0=mult, op1=add
                ).then_inc(dve_sem, 1)

    return nc


def _build_v2(reps, bench):
    """Single-engine (GPSIMD) minimal-instruction variant.

    Whole per-core slice in SBUF at once: xr,xi [128, 8192] (32 KB/partition
    each), out [128, 16384] (64 KB/partition).  4 elementwise ops, the two
    accumulating ops alias in1 == out:
        out[0::2] = xi*b ; out[1::2] = xr*d
        out[0::2] = xr*a + out[0::2] ; out[1::2] = xi*a + out[1::2]
    """
    f32 = mybir.dt.float32
    mult = mybir.AluOpType.mult
    add = mybir.AluOpType.add

    nc = bass.Bass()
    xr, xi, coef, out, out_small = _io(nc, bench)

    W = NT * BATCH  # 8192
    xr_v = xr[:, :].rearrange("(t p) w -> p t w", p=P)
    xi_v = xi[:, :].rearrange("(t p) w -> p t w", p=P)
    out_v = out[:, :].rearrange("(t p) w -> p t w", p=P)

    with (
        nc.sbuf_tensor([P, 3], f32) as coef_t,
        nc.sbuf_tensor([P, W], f32) as xr_b,
        nc.sbuf_tensor([P, W], f32) as xi_b,
        nc.sbuf_tensor([P, W], f32) as tmp_b,
        nc.sbuf_tensor([P, 2 * W], f32) as out_b,
        nc.semaphore() as ld_sem,
        nc.semaphore() as st_sem,
        nc.Block() as block,
    ):
        a_ap = coef_t[:, 0:1]
        b_ap = coef_t[:, 1:2]
        d_ap = coef_t[:, 2:3]
        o_ev = out_b[:, 0::2]
        o_od = out_b[:, 1::2]

        @block.gpsimd
        def _(g):
            g.dma_start(coef_t[:, :], coef[:, :]).then_inc(ld_sem, 16)
            xr_b3 = xr_b[:, :].rearrange("p (t w) -> p t w", t=NT)
            xi_b3 = xi_b[:, :].rearrange("p (t w) -> p t w", t=NT)
            out_b3 = out_b[:, :].rearrange("p (t w) -> p t w", t=NT)
            for r in range(reps):
                g.dma_start(xr_b3, xr_v).then_inc(ld_sem, 16)
                g.dma_start(xi_b3, xi_v).then_inc(ld_sem, 16)
                g.wait_ge(ld_sem, 16 + 32 * (r + 1))
                # Pool rejects scalar_tensor_tensor in this walrus build, so
                # build each plane with ts + ts + aliasing tt-add (6 ops).
                nc.gpsimd.tensor_scalar_mul(o_ev, xi_b[:, :], b_ap)
                nc.gpsimd.tensor_scalar_mul(tmp_b[:, :], xr_b[:, :], a_ap)
                nc.gpsimd.tensor_tensor(o_ev, tmp_b[:, :], o_ev, op=add)
                nc.gpsimd.tensor_scalar_mul(o_od, xr_b[:, :], d_ap)
                nc.gpsimd.tensor_scalar_mul(tmp_b[:, :], xi_b[:, :], a_ap)
                nc.gpsimd.tensor_tensor(o_od, tmp_b[:, :], o_od, op=add)
                g.dma_start(out_v, out_b3).then_inc(st_sem, 16)
                g.wait_ge(st_sem, 16 * (r + 1))
            if bench:
                g.dma_start(out_small[:, :], coef_t[:, :]).then_inc(st_sem, 16)
                g.wait_ge(st_sem, 16 * reps + 16)

    return nc


def _build_v3(reps, bench):
    """Three-engine minimal-instruction variant: SP does DMA, ACT does the
    two prefills (strided dest), DVE does the two accumulating STTs."""
    f32 = mybir.dt.float32
    mult = mybir.AluOpType.mult
    add = mybir.AluOpType.add

    nc = bass.Bass()
    xr, xi, coef, out, out_small = _io(nc, bench)

    W = NT * BATCH
    xr_v = xr[:, :].rearrange("(t p) w -> p t w", p=P)
    xi_v = xi[:, :].rearrange("(t p) w -> p t w", p=P)
    out_v = out[:, :].rearrange("(t p) w -> p t w", p=P)

    with (
        nc.sbuf_tensor([P, 3], f32) as coef_t,
        nc.sbuf_tensor([P, W], f32) as xr_b,
        nc.sbuf_tensor([P, W], f32) as xi_b,
        nc.sbuf_tensor([P, 2 * W], f32) as out_b,
        nc.semaphore() as ld_sem,
        nc.semaphore() as act_sem,
        nc.semaphore() as dve_sem,
        nc.semaphore() as st_sem,
        nc.Block() as block,
    ):
        a_ap = coef_t[:, 0:1]
        b_ap = coef_t[:, 1:2]
        d_ap = coef_t[:, 2:3]
        o_ev = out_b[:, 0::2]
        o_od = out_b[:, 1::2]

        @block.sync
        def _(sync):
            sync.dma_start(coef_t[:, :], coef[:, :]).then_inc(ld_sem, 16)
            xr_b3 = xr_b[:, :].rearrange("p (t w) -> p t w", t=NT)
            xi_b3 = xi_b[:, :].rearrange("p (t w) -> p t w", t=NT)
            out_b3 = out_b[:, :].rearrange("p (t w) -> p t w", t=NT)
            for r in range(reps):
                sync.dma_start(xr_b3, xr_v).then_inc(ld_sem, 16)
                sync.dma_start(xi_b3, xi_v).then_inc(ld_sem, 16)
                sync.wait_ge(dve_sem, 2 * (r + 1))
                sync.dma_start(out_v, out_b3).then_inc(st_sem, 16)
            if bench:
                sync.wait_ge(st_sem, 16 * reps)
                sync.dma_start(out_small[:, :], coef_t[:, :]).then_inc(st_sem, 16)

        @block.scalar
        def _(scalar):
            for r in range(reps):
                scalar.wait_ge(ld_sem, 16 + 32 * (r + 1))
                if r:
                    scalar.wait_ge(st_sem, 16 * r)  # out_b free again
                nc.scalar.mul(o_ev, xi_b[:, :], mul=b_ap).then_inc(act_sem, 1)
                nc.scalar.mul(o_od, xr_b[:, :], mul=d_ap).then_inc(act_sem, 1)

        @block.vector
        def _(vector):
            for r in range(reps):
                vector.wait_ge(act_sem, 2 * r + 1)
                nc.vector.scalar_tensor_tensor(
                    o_ev, xr_b[:, :], a_ap, o_ev, op0=mult, op1=add
                ).then_inc(dve_sem, 1)
                vector.wait_ge(act_sem, 2 * r + 2)
                nc.vector.scalar_tensor_tensor(
                    o_od, xi_b[:, :], a_ap, o_od, op0=mult, op1=add
                ).then_inc(dve_sem, 1)

    return nc


def _build_v1(reps, bench):
    """Pipelined 4-tile variant (classic double-buffered roofline design)."""
    f32 = mybir.dt.float32
    mult = mybir.AluOpType.mult
    add = mybir.AluOpType.add

    nc = bass.Bass()
    xr, xi, coef, out, out_small = _io(nc, bench)

    xr_v = xr[:, :].rearrange("(t p) w -> t p w", p=P)
    xi_v = xi[:, :].rearrange("(t p) w -> t p w", p=P)
    out_v = out[:, :].rearrange("(t p) w -> t p w", p=P)

    with (
        nc.sbuf_tensor([P, 3], f32) as coef_t,
        nc.sbuf_tensor([P, NBUF * BATCH], f32) as xr_b,
        nc.sbuf_tensor([P, NBUF * BATCH], f32) as xi_b,
        nc.sbuf_tensor([P, NBUF * BATCH], f32) as t1_b,
        nc.sbuf_tensor([P, NBUF * BATCH], f32) as t2_b,
        nc.sbuf_tensor([P, NBUF * 2 * BATCH], f32) as out_b,
        nc.semaphore() as ld_sem,     # +16 per load DMA (coef + 2 per tile)
        nc.semaphore() as act_sem,    # +1 per ACT op (2 per tile)
        nc.semaphore() as dve_sem,    # +1 per DVE op (2 per tile)
        nc.semaphore() as st_sem,     # +16 per store DMA (1 per tile)
        nc.Block() as block,
    ):
        a_ap = coef_t[:, 0:1]
        b_ap = coef_t[:, 1:2]
        d_ap = coef_t[:, 2:3]

        def xrb(i):
            return xr_b[:, i * BATCH : (i + 1) * BATCH]

        def xib(i):
            return xi_b[:, i * BATCH : (i + 1) * BATCH]

        def t1b(i):
            return t1_b[:, i * BATCH : (i + 1) * BATCH]

        def t2b(i):
            return t2_b[:, i * BATCH : (i + 1) * BATCH]

        def outb(i):
            return out_b[:, i * 2 * BATCH : (i + 1) * 2 * BATCH]

        G = reps * NT  # total tile iterations (DRAM tile index = g % NT)
        st_base = 16 if bench else 0  # bench marker store bumps st_sem once

        def loads(sync, g):
            i, t = g % NBUF, g % NT
            sync.dma_start(xrb(i), xr_v[t, :, :]).then_inc(ld_sem, 16)
            sync.dma_start(xib(i), xi_v[t, :, :]).then_inc(ld_sem, 16)

        @block.sync
        def _(sync):
            sync.dma_start(coef_t[:, :], coef[:, :]).then_inc(ld_sem, 16)
            if bench:
                # tiny marker output so the bench NEFF has a valid external out
                sync.wait_ge(ld_sem, 16)
                sync.dma_start(out_small[:, :], coef_t[:, :]).then_inc(st_sem, 16)
            for g in range(min(NBUF, G)):  # prefetch
                loads(sync, g)
            for g in range(G):
                nxt = g + NBUF
                if nxt < G:
                    # buffers for `nxt` are free once ACT+DVE finished tile g
                    sync.wait_ge(act_sem, 2 * (g + 1))
                    sync.wait_ge(dve_sem, 2 * (g + 1))
                    loads(sync, nxt)
                sync.wait_ge(dve_sem, 2 * (g + 1))
                sync.dma_start(out_v[g % NT, :, :], outb(g % NBUF)).then_inc(st_sem, 16)

        @block.scalar
        def _(scalar):
            for g in range(G):
                i = g % NBUF
                scalar.wait_ge(ld_sem, 16 + 32 * (g + 1))
                if g >= NBUF:
                    # t1/t2 buffers free once DVE finished tile g-NBUF
                    scalar.wait_ge(dve_sem, 2 * (g - NBUF + 1))
                nc.scalar.mul(t1b(i), xib(i), mul=b_ap).then_inc(act_sem, 1)
                nc.scalar.mul(t2b(i), xrb(i), mul=d_ap).then_inc(act_sem, 1)

        @block.vector
        def _(vector):
            for g in range(G):
                i = g % NBUF
                vector.wait_ge(act_sem, 2 * (g + 1))
                if g >= NBUF:
                    # out buffer free once store of tile g-NBUF completed
                    vector.wait_ge(st_sem, st_base + 16 * (g - NBUF + 1))
                ob = outb(i)
                nc.vector.scalar_tensor_tensor(
                    ob[:, 0::2], xrb(i), a_ap, t1b(i), op0=mult, op1=add
                ).then_inc(dve_sem, 1)
                nc.vector.scalar_tensor_tensor(
                    ob[:, 1::2], xib(i), a_ap, t2b(i), op0=mult, op1=add
                ).then_inc(dve_sem, 1)

    return nc


def _coef_for_core(i, c, s):
    if i < 4:
        return (1.0, 0.0, 0.0)
    if i < 6:
        return (c, s, -s)  # phase exp(-i ang): (c - i s)(xr + i xi)
    return (c, -s, s)      # phase exp(+i ang)


def _run(x_real, x_imag, angle, trace=False, reps=1, variant=None):
    variant = variant or VARIANT
    nc = _build(reps=reps, variant=variant)
    ang = 0.5 * float(np.asarray(angle).reshape(-1)[0])
    c, s = math.cos(ang), math.sin(ang)

    xr = np.ascontiguousarray(np.asarray(x_real, dtype=np.float32))
    xi = np.ascontiguousarray(np.asarray(x_imag, dtype=np.float32))

    packed = variant in ("v9", "v10")
    in_maps = []
    for i in range(N_CORES):
        a_, b_, d_ = _coef_for_core(i, c, s)
        if variant in ("v11", "v13"):
            xin = np.empty((2 * ROWS, BATCH), np.float32)
            xin[:ROWS] = xi[i * ROWS : (i + 1) * ROWS]
            xin[ROWS:] = xr[i * ROWS : (i + 1) * ROWS]
            coef = np.empty((P, 3), np.float32)
            coef[:, 0] = a_
            coef[:, 1] = b_
            coef[:, 2] = d_
            in_maps.append({"xin": xin, "coef": coef})
            continue
        if packed:
            xr_aug = np.empty((ROWS, WA), np.float32)
            xr_aug[:, :BATCH] = xr[i * ROWS : (i + 1) * ROWS]
            xr_aug[:, BATCH] = a_
            xr_aug[:, BATCH + 1] = b_
            xr_aug[:, BATCH + 2] = d_
            xr_aug[:, BATCH + 3] = 0.0
            in_maps.append(
                {"xr": xr_aug, "xi": xi[i * ROWS : (i + 1) * ROWS]}
            )
            continue
        coef = np.empty((P, 3), np.float32)
        coef[:, 0] = a_
        coef[:, 1] = b_
        coef[:, 2] = d_
        in_maps.append(
            {
                "xr": xr[i * ROWS : (i + 1) * ROWS],
                "xi": xi[i * ROWS : (i + 1) * ROWS],
                "coef": coef,
            }
        )

    kw = {}
    if trace:
        kw = dict(trace=True, trace_cores=list(range(N_CORES)))
    res = run_bass_kernel_spmd(nc, in_maps, core_ids=list(range(N_CORES)), **kw)

    out = np.empty((D, 2 * BATCH), np.float32)
    for i in range(N_CORES):
        out[i * ROWS : (i + 1) * ROWS] = res.results[i]["out"]
    return out.view(np.complex64), res


def kernel(x_real, x_imag, angle):
    out, _ = _run(x_real, x_imag, angle)
    return out
